# revision 5
# baseline (speedup 1.0000x reference)
"""AAMSoftmax (norm-free) loss head on 8 Trainium2 NeuronCores.

Math (reference):
    norm    = ||x_b||                                  [B, 1]
    xn      = x / max(norm, eps); wn = W / max(||W_row||, eps)
    cosine  = xn @ wn.T                                [B, OUT]
    phi     = cos(theta + m) = cosine*cos(m) - sine*sin(m)
    out     = norm * where(onehot(label) & cosine > 0, phi, cosine)
    returns (out, wn)

Key identity: norm * cosine == x @ wn.T exactly (norm >> eps), so the big
[B, OUT] tensor is a single matmul of the *unnormalized* x against the
row-normalized weights; the margin substitution only touches B entries
(one per row, at column label_b). Each core computes its 1/8 slab of
out/wn plus the per-row substituted value v_b = norm_b * (cl>0 ? phi : cl)
(cl = cosine at the label column, computed from host-gathered weight rows).
The host assembles slabs and writes v at the B label positions.

Sharding: W rows (out_features) split 8 ways, 8000 rows/core padded to
8192; x/wlab/m replicated.
"""

import functools

import numpy as np

B, IN, OUT = 1024, 512, 64000
NCORES = 8
SLAB = OUT // NCORES  # 8000
PAD = 8192            # per-core weight rows, padded for uniform 128/512 tiling
EPS = 1e-12
HALF_PI = 1.5707963267948966


@functools.lru_cache(maxsize=None)
def _build(batch=B, in_features=IN, pad=PAD, reps=1):
    import concourse.bass as bass
    import concourse.mybir as mybir
    import concourse.tile as tile
    from concourse import bacc
    from concourse.bass import ds, ts
    from concourse.masks import make_identity
    from contextlib import ExitStack

    f32 = mybir.dt.float32
    Sin = mybir.ActivationFunctionType.Sin
    mult = mybir.AluOpType.mult
    add = mybir.AluOpType.add
    is_gt = mybir.AluOpType.is_gt

    KT = in_features // 128   # contraction chunks
    NBT = batch // 128        # batch tiles
    NOG = pad // 512          # out-feature groups of 512

    nc = bacc.Bacc("TRN2", target_bir_lowering=False, debug=False)

    x_ext = nc.dram_tensor("x", [batch, in_features], f32, kind="ExternalInput")
    w_ext = nc.dram_tensor("w", [pad, in_features], f32, kind="ExternalInput")
    wl_ext = nc.dram_tensor("wlab", [batch, in_features], f32, kind="ExternalInput")
    m_ext = nc.dram_tensor("m", [1, 1], f32, kind="ExternalInput")
    out_ext = nc.dram_tensor("out", [batch, pad], f32, kind="ExternalOutput")
    wn_ext = nc.dram_tensor("wn", [pad, in_features], f32, kind="ExternalOutput")
    v_ext = nc.dram_tensor("v", [batch, 1], f32, kind="ExternalOutput")
    trig_dram = nc.dram_tensor("trig_scratch", [1, 2], f32)

    with ExitStack() as ctx:
        tc = ctx.enter_context(tile.TileContext(nc))
        singles = ctx.enter_context(tc.tile_pool(name="singles", bufs=1))
        xin = ctx.enter_context(tc.tile_pool(name="xin", bufs=3))
        small = ctx.enter_context(tc.tile_pool(name="small", bufs=4))
        wpool = ctx.enter_context(tc.tile_pool(name="wpool", bufs=3))
        wnpool = ctx.enter_context(tc.tile_pool(name="wnpool", bufs=3))
        wntp = ctx.enter_context(tc.tile_pool(name="wntp", bufs=2))
        outp = ctx.enter_context(tc.tile_pool(name="outp", bufs=4))
        psum_t = ctx.enter_context(tc.tile_pool(name="psum_t", bufs=2, space="PSUM"))
        psum_mm = ctx.enter_context(tc.tile_pool(name="psum_mm", bufs=4, space="PSUM"))

        def emit_body():
            identity = singles.tile([128, 128], f32)
            make_identity(nc, identity)

            # cos(m), -sin(m) -> broadcast to all partitions via a DRAM bounce
            m_sb = singles.tile([1, 1], f32)
            nc.sync.dma_start(out=m_sb, in_=m_ext[:, :])
            trig = singles.tile([1, 2], f32)
            halfpi = singles.tile([1, 1], f32)
            nc.vector.memset(halfpi, HALF_PI)
            nc.scalar.activation(trig[:, 0:1], m_sb, Sin, scale=-1.0)    # -sin(m)
            nc.scalar.activation(trig[:, 1:2], m_sb, Sin, bias=halfpi)   # cos(m)
            nc.sync.dma_start(out=trig_dram[:, :], in_=trig)
            trig_bc = singles.tile([128, 2], f32)
            nc.sync.dma_start(out=trig_bc, in_=trig_dram[:, :].to_broadcast([128, 2]))
            negsinm = trig_bc[:, 0:1]
            cosm = trig_bc[:, 1:2]

            # x^T, kept resident: [128, KT, batch]
            xT = singles.tile([128, KT, batch], f32)

            for bt in range(NBT):
                xt = xin.tile([128, in_features], f32)
                nc.sync.dma_start(out=xt, in_=x_ext[ts(bt, 128), :])
                wl = xin.tile([128, in_features], f32)
                nc.sync.dma_start(out=wl, in_=wl_ext[ts(bt, 128), :])

                scr = xin.tile([128, in_features], f32, tag="scr")
                ssx = small.tile([128, 1], f32)
                nc.vector.scalar_tensor_tensor(
                    out=scr, in0=xt, scalar=1.0, in1=xt, op0=mult, op1=mult,
                    accum_out=ssx)
                norm = small.tile([128, 1], f32)
                nc.scalar.sqrt(norm, ssx)
                ssw = small.tile([128, 1], f32)
                nc.vector.scalar_tensor_tensor(
                    out=scr, in0=wl, scalar=1.0, in1=wl, op0=mult, op1=mult,
                    accum_out=ssw)
                wnorm = small.tile([128, 1], f32)
                nc.scalar.sqrt(wnorm, ssw)
                dotv = small.tile([128, 1], f32)
                nc.vector.scalar_tensor_tensor(
                    out=scr, in0=xt, scalar=1.0, in1=wl, op0=mult, op1=mult,
                    accum_out=dotv)

                den = small.tile([128, 1], f32)
                normc = small.tile([128, 1], f32)
                nc.vector.tensor_scalar_max(normc, norm, EPS)
                wnormc = small.tile([128, 1], f32)
                nc.vector.tensor_scalar_max(wnormc, wnorm, EPS)
                nc.vector.tensor_mul(den, normc, wnormc)
                rden = small.tile([128, 1], f32)
                nc.vector.reciprocal(rden, den)
                cl = small.tile([128, 1], f32)
                nc.vector.tensor_mul(cl, dotv, rden)

                mask = small.tile([128, 1], f32)
                nc.vector.tensor_scalar(
                    out=mask, in0=cl, scalar1=0.0, scalar2=None, op0=is_gt)
                sine = small.tile([128, 1], f32)
                nc.vector.tensor_mul(sine, cl, cl)
                nc.vector.tensor_scalar(
                    out=sine, in0=sine, scalar1=-1.0, scalar2=1.0, op0=mult, op1=add)
                nc.vector.tensor_scalar_max(sine, sine, 0.0)
                nc.scalar.sqrt(sine, sine)
                phi = small.tile([128, 1], f32)
                nc.vector.tensor_scalar_mul(phi, cl, cosm)
                nc.vector.scalar_tensor_tensor(
                    out=phi, in0=sine, scalar=negsinm, in1=phi, op0=mult, op1=add)
                dv = small.tile([128, 1], f32)
                nc.vector.tensor_sub(dv, phi, cl)
                vv = small.tile([128, 1], f32)
                nc.vector.scalar_tensor_tensor(
                    out=vv, in0=dv, scalar=mask, in1=cl, op0=mult, op1=add)
                nc.vector.tensor_mul(vv, vv, norm)
                nc.sync.dma_start(out=v_ext[ts(bt, 128), :], in_=vv)

                pst = psum_t.tile([128, KT, 128], f32, tag="pst")
                for k in range(KT):
                    nc.tensor.transpose(pst[:, k, :], xt[:, ts(k, 128)], identity)
                nc.scalar.copy(out=xT[:, :, ts(bt, 128)], in_=pst)

            for og in range(NOG):
                wnt = wntp.tile([128, KT, 512], f32)
                for j in range(4):
                    row0 = og * 512 + j * 128
                    wt = wpool.tile([128, in_features], f32)
                    nc.sync.dma_start(out=wt, in_=w_ext[ds(row0, 128), :])
                    wscr = wpool.tile([128, in_features], f32, tag="wscr")
                    ssw2 = small.tile([128, 1], f32, tag="ssw2")
                    nc.vector.scalar_tensor_tensor(
                        out=wscr, in0=wt, scalar=1.0, in1=wt, op0=mult, op1=mult,
                        accum_out=ssw2)
                    wn_norm = small.tile([128, 1], f32, tag="wn_norm")
                    nc.scalar.sqrt(wn_norm, ssw2)
                    nc.vector.tensor_scalar_max(wn_norm, wn_norm, EPS)
                    winv = small.tile([128, 1], f32, tag="winv")
                    nc.vector.reciprocal(winv, wn_norm)
                    wnt_sb = wnpool.tile([128, in_features], f32)
                    nc.vector.tensor_scalar_mul(wnt_sb, wt, winv)
                    nc.sync.dma_start(out=wn_ext[ds(row0, 128), :], in_=wnt_sb)
                    pst2 = psum_t.tile([128, KT, 128], f32, tag="pst")
                    for k in range(KT):
                        nc.tensor.transpose(
                            pst2[:, k, :], wnt_sb[:, ts(k, 128)], identity)
                    nc.scalar.copy(out=wnt[:, :, ts(j, 128)], in_=pst2)

                for bt in range(NBT):
                    ps = psum_mm.tile([128, 512], f32)
                    for k in range(KT):
                        nc.tensor.matmul(
                            ps, lhsT=xT[:, k, ts(bt, 128)], rhs=wnt[:, k, :],
                            start=(k == 0), stop=(k == KT - 1))
                    ot = outp.tile([128, 512], f32)
                    if bt % 2 == 0:
                        nc.scalar.copy(out=ot, in_=ps)
                    else:
                        nc.vector.tensor_copy(out=ot, in_=ps)
                    nc.sync.dma_start(
                        out=out_ext[ts(bt, 128), ds(og * 512, 512)], in_=ot)

        if reps == 1:
            emit_body()
        else:
            with tc.For_i(0, reps, 1):
                emit_body()

    nc.compile()
    return nc


def _run(nc, in_maps, trace=False):
    from concourse.bass_utils import run_bass_kernel_spmd

    return run_bass_kernel_spmd(
        nc, in_maps, core_ids=list(range(len(in_maps))), trace=trace)


def kernel(**inputs):
    x = np.asarray(inputs["x"], dtype=np.float32)
    label = np.asarray(inputs["label"]).astype(np.int64)
    weight = np.asarray(inputs["weight"], dtype=np.float32)
    m = np.asarray(inputs["m"], dtype=np.float32).reshape(1, 1)

    nc = _build()

    wlab = np.ascontiguousarray(weight[label])  # [B, IN] host gather (sharding prep)
    in_maps = []
    for c in range(NCORES):
        wpad = np.zeros((PAD, IN), np.float32)
        wpad[:SLAB] = weight[c * SLAB:(c + 1) * SLAB]
        in_maps.append({"x": x, "w": wpad, "wlab": wlab, "m": m})

    res = _run(nc, in_maps).results

    out = np.concatenate([r["out"][:, :SLAB] for r in res], axis=1)
    wn = np.concatenate([r["wn"][:SLAB] for r in res], axis=0)
    out[np.arange(B), label] = res[0]["v"][:, 0]
    return out, wn


# revision 8
# speedup vs baseline: 3.4768x; 3.4768x over previous
"""AAMSoftmax (norm-free) loss head on 8 Trainium2 NeuronCores.

Math (reference):
    norm    = ||x_b||                                  [B, 1]
    xn      = x / max(norm, eps); wn = W / max(||W_row||, eps)
    cosine  = xn @ wn.T                                [B, OUT]
    phi     = cos(theta + m) = cosine*cos(m) - sine*sin(m)
    out     = norm * where(onehot(label) & cosine > 0, phi, cosine)
    returns (out, wn)

Key identity: norm * cosine == x @ wn.T exactly (norm >> eps), so the big
[B, OUT] tensor is a single matmul of the *unnormalized* x against the
row-normalized weights; the margin substitution only touches B entries
(one per row, at column label_b). Each core computes its 1/8 slab of
out/wn plus the per-row substituted value v_b = norm_b * (cl>0 ? phi : cl)
(cl = cosine at the label column, computed from host-gathered weight rows).
The host assembles slabs and writes v at the B label positions.

Sharding: W rows (out_features) split 8 ways, 8000 rows/core padded to
8192; x/wlab/m replicated.
"""

import functools

import numpy as np

B, IN, OUT = 1024, 512, 64000
NCORES = 8
SLAB = OUT // NCORES  # 8000
PAD = 8192            # per-core weight rows, padded for uniform 128/512 tiling
EPS = 1e-12
HALF_PI = 1.5707963267948966

MM = "f32"     # matmul operand mode: f32 | f32r | bf16
STORE = "f32"  # out/wn DRAM dtype: f32 | bf16


@functools.lru_cache(maxsize=None)
def _build(batch=B, in_features=IN, pad=PAD, reps=1, mm=MM, store=STORE):
    import concourse.bass as bass
    import concourse.mybir as mybir
    import concourse.tile as tile
    from concourse import bacc
    from concourse.bass import ds, ts
    from concourse.masks import make_identity
    from contextlib import ExitStack

    f32 = mybir.dt.float32
    bf16 = mybir.dt.bfloat16
    f32r = mybir.dt.float32r
    Sin = mybir.ActivationFunctionType.Sin
    mult = mybir.AluOpType.mult
    add = mybir.AluOpType.add
    is_gt = mybir.AluOpType.is_gt

    mmdt = {"f32": f32, "f32r": f32r, "bf16": bf16}[mm]  # SBUF storage dtype
    stdt = {"f32": f32, "bf16": bf16}[store]

    KT = in_features // 128   # contraction chunks
    NBT = batch // 128        # batch tiles
    NOG = pad // 512          # out-feature groups of 512

    nc = bacc.Bacc("TRN2", target_bir_lowering=False, debug=False)

    x_ext = nc.dram_tensor("x", [batch, in_features], f32, kind="ExternalInput")
    w_ext = nc.dram_tensor("w", [pad, in_features], f32, kind="ExternalInput")
    wl_ext = nc.dram_tensor("wlab", [batch, in_features], f32, kind="ExternalInput")
    m_ext = nc.dram_tensor("m", [1, 1], f32, kind="ExternalInput")
    out_ext = nc.dram_tensor("out", [batch, pad], stdt, kind="ExternalOutput")
    wn_ext = nc.dram_tensor("wn", [pad, in_features], stdt, kind="ExternalOutput")
    v_ext = nc.dram_tensor("v", [batch, 1], f32, kind="ExternalOutput")
    trig_dram = nc.dram_tensor("trig_scratch", [1, 2], f32)

    with ExitStack() as ctx:
        tc = ctx.enter_context(tile.TileContext(nc))
        singles = ctx.enter_context(tc.tile_pool(name="singles", bufs=1))
        xin = ctx.enter_context(tc.tile_pool(name="xin", bufs=3))
        small = ctx.enter_context(tc.tile_pool(name="small", bufs=4))
        wpool = ctx.enter_context(tc.tile_pool(name="wpool", bufs=3))
        wnpool = ctx.enter_context(tc.tile_pool(name="wnpool", bufs=3))
        wntp = ctx.enter_context(tc.tile_pool(name="wntp", bufs=2))
        outp = ctx.enter_context(tc.tile_pool(name="outp", bufs=4))
        psum_t = ctx.enter_context(tc.tile_pool(name="psum_t", bufs=2, space="PSUM"))
        psum_mm = ctx.enter_context(tc.tile_pool(name="psum_mm", bufs=4, space="PSUM"))

        def emit_body():
            identity = singles.tile([128, 128], f32)
            make_identity(nc, identity)

            # cos(m), -sin(m) -> broadcast to all partitions via a DRAM bounce
            m_sb = singles.tile([1, 1], f32)
            nc.sync.dma_start(out=m_sb, in_=m_ext[:, :])
            trig = singles.tile([1, 2], f32)
            halfpi = singles.tile([1, 1], f32)
            nc.vector.memset(halfpi, HALF_PI)
            nc.scalar.activation(trig[:, 0:1], m_sb, Sin, scale=-1.0)    # -sin(m)
            nc.scalar.activation(trig[:, 1:2], m_sb, Sin, bias=halfpi)   # cos(m)
            nc.sync.dma_start(out=trig_dram[:, :], in_=trig)
            trig_bc = singles.tile([128, 2], f32)
            nc.sync.dma_start(out=trig_bc, in_=trig_dram[:, :].to_broadcast([128, 2]))
            negsinm = trig_bc[:, 0:1]
            cosm = trig_bc[:, 1:2]

            # x^T, kept resident: [128, KT, batch]
            xT = singles.tile([128, KT, batch], mmdt)

            for bt in range(NBT):
                xt = xin.tile([128, in_features], f32)
                nc.sync.dma_start(out=xt, in_=x_ext[ts(bt, 128), :])
                wl = xin.tile([128, in_features], f32)
                nc.sync.dma_start(out=wl, in_=wl_ext[ts(bt, 128), :])

                scr = xin.tile([128, in_features], f32, tag="scr")
                ssx = small.tile([128, 1], f32)
                nc.vector.scalar_tensor_tensor(
                    out=scr, in0=xt, scalar=1.0, in1=xt, op0=mult, op1=mult,
                    accum_out=ssx)
                norm = small.tile([128, 1], f32)
                nc.scalar.sqrt(norm, ssx)
                ssw = small.tile([128, 1], f32)
                nc.vector.scalar_tensor_tensor(
                    out=scr, in0=wl, scalar=1.0, in1=wl, op0=mult, op1=mult,
                    accum_out=ssw)
                wnorm = small.tile([128, 1], f32)
                nc.scalar.sqrt(wnorm, ssw)
                dotv = small.tile([128, 1], f32)
                nc.vector.scalar_tensor_tensor(
                    out=scr, in0=xt, scalar=1.0, in1=wl, op0=mult, op1=mult,
                    accum_out=dotv)

                den = small.tile([128, 1], f32)
                normc = small.tile([128, 1], f32)
                nc.vector.tensor_scalar_max(normc, norm, EPS)
                wnormc = small.tile([128, 1], f32)
                nc.vector.tensor_scalar_max(wnormc, wnorm, EPS)
                nc.vector.tensor_mul(den, normc, wnormc)
                rden = small.tile([128, 1], f32)
                nc.vector.reciprocal(rden, den)
                cl = small.tile([128, 1], f32)
                nc.vector.tensor_mul(cl, dotv, rden)

                mask = small.tile([128, 1], f32)
                nc.vector.tensor_scalar(
                    out=mask, in0=cl, scalar1=0.0, scalar2=None, op0=is_gt)
                sine = small.tile([128, 1], f32)
                nc.vector.tensor_mul(sine, cl, cl)
                nc.vector.tensor_scalar(
                    out=sine, in0=sine, scalar1=-1.0, scalar2=1.0, op0=mult, op1=add)
                nc.vector.tensor_scalar_max(sine, sine, 0.0)
                nc.scalar.sqrt(sine, sine)
                phi = small.tile([128, 1], f32)
                nc.vector.tensor_scalar_mul(phi, cl, cosm)
                nc.vector.scalar_tensor_tensor(
                    out=phi, in0=sine, scalar=negsinm, in1=phi, op0=mult, op1=add)
                dv = small.tile([128, 1], f32)
                nc.vector.tensor_sub(dv, phi, cl)
                vv = small.tile([128, 1], f32)
                nc.vector.scalar_tensor_tensor(
                    out=vv, in0=dv, scalar=mask, in1=cl, op0=mult, op1=add)
                nc.vector.tensor_mul(vv, vv, norm)
                nc.sync.dma_start(out=v_ext[ts(bt, 128), :], in_=vv)

                pst = psum_t.tile([128, KT, 128], f32, tag="pst")
                for k in range(KT):
                    nc.tensor.transpose(pst[:, k, :], xt[:, ts(k, 128)], identity)
                nc.scalar.copy(out=xT[:, :, ts(bt, 128)], in_=pst)

            for og in range(NOG):
                wnt = wntp.tile([128, KT, 512], mmdt)
                for j in range(4):
                    row0 = og * 512 + j * 128
                    wt = wpool.tile([128, in_features], f32)
                    nc.sync.dma_start(out=wt, in_=w_ext[ds(row0, 128), :])
                    wscr = wpool.tile([128, in_features], f32, tag="wscr")
                    ssw2 = small.tile([128, 1], f32, tag="ssw2")
                    nc.vector.scalar_tensor_tensor(
                        out=wscr, in0=wt, scalar=1.0, in1=wt, op0=mult, op1=mult,
                        accum_out=ssw2)
                    wn_norm = small.tile([128, 1], f32, tag="wn_norm")
                    nc.scalar.sqrt(wn_norm, ssw2)
                    nc.vector.tensor_scalar_max(wn_norm, wn_norm, EPS)
                    winv = small.tile([128, 1], f32, tag="winv")
                    nc.vector.reciprocal(winv, wn_norm)
                    wnt_sb = wnpool.tile([128, in_features], stdt)
                    nc.vector.tensor_scalar_mul(wnt_sb, wt, winv)
                    nc.sync.dma_start(out=wn_ext[ds(row0, 128), :], in_=wnt_sb)
                    pst2 = psum_t.tile([128, KT, 128], f32, tag="pst")
                    for k in range(KT):
                        nc.tensor.transpose(
                            pst2[:, k, :], wnt_sb[:, ts(k, 128)], identity)
                    nc.scalar.copy(out=wnt[:, :, ts(j, 128)], in_=pst2)

                for bt in range(NBT):
                    ps = psum_mm.tile([128, 512], f32)
                    for k in range(KT):
                        nc.tensor.matmul(
                            ps, lhsT=xT[:, k, ts(bt, 128)], rhs=wnt[:, k, :],
                            start=(k == 0), stop=(k == KT - 1))
                    ot = outp.tile([128, 512], stdt)
                    if bt % 2 == 0:
                        nc.scalar.copy(out=ot, in_=ps)
                    else:
                        nc.vector.tensor_copy(out=ot, in_=ps)
                    nc.sync.dma_start(
                        out=out_ext[ts(bt, 128), ds(og * 512, 512)], in_=ot)

        if reps == 1:
            emit_body()
        else:
            with tc.For_i(0, reps, 1):
                emit_body()

    nc.compile()
    return nc


def _run(nc, in_maps, trace=False):
    from concourse.bass_utils import run_bass_kernel_spmd

    return run_bass_kernel_spmd(
        nc, in_maps, core_ids=list(range(len(in_maps))), trace=trace)


def _make_in_maps(x, label, weight, m):
    wlab = np.ascontiguousarray(weight[label])  # [B, IN] host gather
    in_maps = []
    for c in range(NCORES):
        wpad = np.zeros((PAD, IN), np.float32)
        wpad[:SLAB] = weight[c * SLAB:(c + 1) * SLAB]
        in_maps.append({"x": x, "w": wpad, "wlab": wlab, "m": m})
    return in_maps


def _assemble(res, label):
    out = np.concatenate(
        [np.asarray(r["out"][:, :SLAB], dtype=np.float32) for r in res], axis=1)
    wn = np.concatenate(
        [np.asarray(r["wn"][:SLAB], dtype=np.float32) for r in res], axis=0)
    out[np.arange(B), label] = res[0]["v"][:, 0]
    return out, wn


def kernel(**inputs):
    x = np.asarray(inputs["x"], dtype=np.float32)
    label = np.asarray(inputs["label"]).astype(np.int64)
    weight = np.asarray(inputs["weight"], dtype=np.float32)
    m = np.asarray(inputs["m"], dtype=np.float32).reshape(1, 1)

    nc = _build()
    res = _run(nc, _make_in_maps(x, label, weight, m)).results
    return _assemble(res, label)
